# revision 27
# baseline (speedup 1.0000x reference)
"""Neighbourhood attention block (7x7 clamped window) on 8 Trainium2 cores.

Sharding: (batch, head-pair) tensor parallel. Core c handles batch b = c//4
and heads (2*(c%4), 2*(c%4)+1). Each core computes q/k/v projections for its
two heads, neighbourhood attention, and a partial output projection
y_partial = ao @ w_out_slice^T (bf16). Host sums the 4 partials per batch.

Attention layout: scoresT [key, query] tiles so PV needs no transposes.
Keys are chunked 2 image rows (128 tokens) per chunk; each chunk is matched
against the 8 query rows that can see it (512 queries, one N=512 matmul).
Masking is multiplicative 0/1 after exp (bf16), so invalid keys contribute 0
to both PV numerator and the denominator (a ones-column appended to V).

v2 performance structure:
- everything bf16 on the matmul paths (same PE rate at N>=256, half DMA/DVE)
- x DMA split into 8 column chunks so projections start immediately
- score tiles PAIRED into one 2-bank PSUM tile; ONE exp ACTIVATE per pair
  (the ~352-cycle ACT fixed cost was 40% of scalar time)
- softmax normalization batched per unit: PV emits unnormalized numerators
  (copied to aoU) and denominators (copied to a [1, S] f32 row); one
  reciprocal_approx_fast + one partition_broadcast + 4 wide muls replace
  36 tiny single-partition RECIPROCALs (1.5us each!) and 36 broadcasts.
"""
import os
import numpy as np
import ml_dtypes
from contextlib import ExitStack

_PHASES = os.environ.get("KERNEL_PHASES", "123")  # debug bisect knob

import concourse.bass as bass
import concourse.bacc as bacc
import concourse.tile as tile
import concourse.mybir as mybir
from concourse.bass_utils import run_bass_kernel_spmd
from concourse.masks import make_identity

F32 = mybir.dt.float32
BF16 = mybir.dt.bfloat16

B, H, W, D = 2, 64, 64, 512
DH, NH = 64, 8
S = H * W              # 4096 tokens per batch
KER = 7
SCALE = DH ** -0.5     # 0.125
NCORES = 8

# ---------------------------------------------------------------- geometry

def _sh(r):            # clamped window start (rows); same formula for cols
    return min(max(r - KER // 2, 0), H - KER)


def _chunks_of_row(r):  # key chunks (2 rows each) seen by query row r
    s = _sh(r)
    return list(range(s // 2, (s + KER + 1) // 2))


def _build_plan():
    """Tiles: scoresT [128 keys of chunk c, qw queries at q0]. Groups: PV
    accumulations [65, qw] covering disjoint query ranges."""
    tiles = []          # dict(c, q0, qw)
    for c in range(32):
        q0r = min(max(2 * c - 3, 0), 56)
        tiles.append(dict(c=c, q0=q0r * 64, qw=512))
    for c in (2, 3):        # query rows 0..2 miss these chunks' main windows
        tiles.append(dict(c=c, q0=0, qw=192))
    for c in (28, 29):      # query rows 61..63
        tiles.append(dict(c=c, q0=61 * 64, qw=192))

    # rows covered by each tile, for resolution
    def covers(t, r):
        return t["q0"] <= r * 64 and (r + 1) * 64 <= t["q0"] + t["qw"]

    groups = []         # dict(q0, qw, rows)
    groups.append(dict(rows=[0]))
    groups.append(dict(rows=[1, 2]))
    for k in range(14):
        r0 = 4 * k + 3
        groups.append(dict(rows=[r0, r0 + 1, r0 + 2, r0 + 3]))
    groups.append(dict(rows=[59, 60, 61, 62]))
    groups.append(dict(rows=[63]))

    for g in groups:
        rows = g["rows"]
        g["q0"] = rows[0] * 64
        g["qw"] = len(rows) * 64
        # chunk -> contiguous row subrange of this group needing it
        chunk_rows = {}
        for r in rows:
            for c in _chunks_of_row(r):
                a, b = chunk_rows.get(c, (r, r))
                chunk_rows[c] = (min(a, r), max(b, r))
        mms = []        # (c, row_a, row_b_inclusive, width)
        for c, (ra, rb) in sorted(chunk_rows.items()):
            mms.append((c, ra, rb, (rb - ra + 1) * 64))
        mms.sort(key=lambda m: -m[3])   # widest (full-group) first for start=True
        assert mms[0][3] == g["qw"], (g, mms)
        # resolve each (c, row range) to probs-tile segments
        segs = []       # (c, tile_i, tile_off, out_off, length)
        for c, ra, rb, _w in mms:
            r = ra
            while r <= rb:
                cand = [i for i, t in enumerate(tiles) if t["c"] == c and covers(t, r)]
                assert cand, (g, c, r)
                ti = cand[0]
                t = tiles[ti]
                # extend run while same tile covers
                r2 = r
                while r2 + 1 <= rb and covers(t, r2 + 1):
                    r2 += 1
                segs.append((c, ti, r * 64 - t["q0"], r * 64 - g["q0"],
                             (r2 - r + 1) * 64))
                r = r2 + 1
        g["segs"] = segs

    # sanity: every (query row, chunk) incidence consumed exactly once
    seen = set()
    for g in groups:
        for c, ti, toff, ooff, ln in g["segs"]:
            for r in range((g["q0"] + ooff) // 64, (g["q0"] + ooff + ln) // 64):
                key = (r, c)
                assert key not in seen, key
                seen.add(key)
    for r in range(H):
        for c in _chunks_of_row(r):
            assert (r, c) in seen, (r, c)

    # masks per tile (0/1), deduped
    starts = np.minimum(np.maximum(np.arange(H) - KER // 2, 0), H - KER)
    valid = (np.arange(H)[None, :] >= starts[:, None]) & \
            (np.arange(H)[None, :] < starts[:, None] + KER)   # [q pos, k pos]

    def tile_mask(t):
        ktok = t["c"] * 128 + np.arange(128)
        qtok = t["q0"] + np.arange(t["qw"])
        m = np.zeros((128, 512), np.float32)
        m[:, :t["qw"]] = (valid[qtok[None, :] // 64, ktok[:, None] // 64]
                          & valid[qtok[None, :] % 64, ktok[:, None] % 64])
        return m

    mask_list, mask_ids = [], {}
    for t in tiles:
        m = tile_mask(t)
        key = m.tobytes()
        if key not in mask_ids:
            mask_ids[key] = len(mask_list)
            mask_list.append(m)
        t["mask_id"] = mask_ids[key]

    # first-need order of tiles over the group sequence, and static pairing
    # of consecutive tiles (each pair shares one 2-bank PSUM tile + one exp
    # + one paired-mask multiply)
    order, seen_t = [], set()
    for g in groups:
        for _c, ti, _to, _oo, _ln in g["segs"]:
            if ti not in seen_t:
                seen_t.add(ti)
                order.append(ti)
    pairs = []          # list of (ti_a, ti_b or None)
    for i in range(0, len(order), 2):
        pairs.append((order[i], order[i + 1] if i + 1 < len(order) else None))
    for pid, (a, b) in enumerate(pairs):
        tiles[a]["pair"] = pid
        tiles[a]["poff"] = 0
        if b is not None:
            tiles[b]["pair"] = pid
            tiles[b]["poff"] = tiles[a]["qw"]

    # paired masks [128, 1024], deduped across pairs
    pm_list, pm_ids = [], {}
    for pid, (a, b) in enumerate(pairs):
        m = np.zeros((128, 1024), np.float32)
        wa = tiles[a]["qw"]
        m[:, :wa] = mask_list[tiles[a]["mask_id"]][:, :wa]
        if b is not None:
            wb = tiles[b]["qw"]
            m[:, wa:wa + wb] = mask_list[tiles[b]["mask_id"]][:, :wb]
        key = m.tobytes()
        if key not in pm_ids:
            pm_ids[key] = len(pm_list)
            pm_list.append(m)
        pairs[pid] = (a, b, pm_ids[key])

    # den-normalization blocks: runs of groups whose cumulative query end is
    # a multiple of 64 and roughly 1k wide -> pipelined reshape/recip
    blocks, start, prev_end = [], 0, 0
    for gi, g in enumerate(groups):
        end = g["q0"] + g["qw"]
        if end - start >= 960:
            blocks.append((gi, start, end))
            start = end
    if start < groups[-1]["q0"] + groups[-1]["qw"]:
        blocks.append((len(groups) - 1, start, groups[-1]["q0"] + groups[-1]["qw"]))
    assert all((e - s) % 64 == 0 for _, s, e in blocks)
    return tiles, groups, np.stack(mask_list), pairs, np.stack(pm_list), blocks


TILES, GROUPS, MASKS, PAIRS, PMASKS, DBLOCKS = _build_plan()
NMASK = len(MASKS)
NPMASK = len(PMASKS)
MAXBW = max((e - s) // 64 for _, s, e in DBLOCKS)

# ---------------------------------------------------------------- device

_NC_CACHE = {}
TRACE = False          # set True (e.g. from test.py) to capture an NTFF profile
TRACE_DIR = None       # persistent dir for NTFF/perfetto artifacts
LAST_RESULTS = None    # BassKernelResults of the most recent kernel() call


def _build_module():
    nc = bacc.Bacc("TRN2", target_bir_lowering=False, debug=False,
                   num_devices=NCORES)
    xT_d = nc.dram_tensor("xT", [128, 4, S], BF16, kind="ExternalInput")
    wq_d = nc.dram_tensor("wq", [128, 4, 128], BF16, kind="ExternalInput")
    wk_d = nc.dram_tensor("wk", [128, 4, 128], BF16, kind="ExternalInput")
    wv_d = nc.dram_tensor("wv", [128, 4, 128], BF16, kind="ExternalInput")
    wo_d = nc.dram_tensor("wo", [128, 512], BF16, kind="ExternalInput")
    mk_d = nc.dram_tensor("masks", [128, NPMASK, 1024], BF16, kind="ExternalInput")
    y_d = nc.dram_tensor("y", [S, D], BF16, kind="ExternalOutput")
    # DRAM scratch for the softmax-denominator partition reshape
    den_d = [nc.dram_tensor(f"den{u}", [1, S], BF16, kind="Internal") for u in range(2)]
    rcp_d = [nc.dram_tensor(f"rcp{u}", [1, S], BF16, kind="Internal") for u in range(2)]

    with tile.TileContext(nc) as tc, ExitStack() as ctx:
        const = ctx.enter_context(tc.tile_pool(name="const", bufs=1))
        wq_t = const.tile([128, 4, 128], BF16, tag="wq")
        nc.sync.dma_start(out=wq_t[:], in_=wq_d[:, :, :])
        wk_t = const.tile([128, 4, 128], BF16, tag="wk")
        nc.sync.dma_start(out=wk_t[:], in_=wk_d[:, :, :])
        wv_t = const.tile([128, 4, 128], BF16, tag="wv")
        nc.sync.dma_start(out=wv_t[:], in_=wv_d[:, :, :])
        wo_t = const.tile([128, 512], BF16, tag="wo")
        nc.sync.dma_start(out=wo_t[:], in_=wo_d[:, :])
        # masks on the gpsimd-triggered queue so they don't sit behind x
        mk_t = const.tile([128, NPMASK, 1024], BF16, tag="mk")
        nc.gpsimd.dma_start(out=mk_t[:], in_=mk_d[:, :, :])
        # x in 8 column chunks so proj can start on chunk 0 asap
        xT_t = const.tile([128, 4, S], BF16, tag="xT")
        for nb in range(8):
            nc.sync.dma_start(out=xT_t[:, :, nb * 512:(nb + 1) * 512],
                              in_=xT_d[:, :, nb * 512:(nb + 1) * 512])

        qT = const.tile([128, S], BF16, tag="qT")      # [2 heads x 64e, tok]
        kT = const.tile([128, S], BF16, tag="kT")
        vT = const.tile([128, S], BF16, tag="vT")
        # V: [tok_in_chunk, chunk, 130]: cols 0:64 u0-e, 64 ones, 65:129 u1-e, 129 ones
        V = const.tile([128, 32, 130], BF16, tag="V")
        nc.gpsimd.memset(V[:], 1.0)
        aoT = const.tile([128, S], BF16, tag="aoT")    # normalized, outproj lhsT
        # unnormalized numerators (rows 0:64) + denominator (row 64)
        aoU0 = const.tile([65, S], BF16, tag="aoU0")
        aoU1 = const.tile([65, S], BF16, tag="aoU1")
        NBLK = len(DBLOCKS)
        denT = const.tile([64, 2 * NBLK, MAXBW], BF16, tag="denT")  # den reshaped
        denF = const.tile([64, 2 * NBLK, MAXBW], F32, tag="denF")
        rcpF = const.tile([64, 2 * NBLK, MAXBW], F32, tag="rcpF")
        rcpB = const.tile([64, 2 * NBLK, MAXBW], BF16, tag="rcpB")
        rcp_row = const.tile([1, 2, S], BF16, tag="rcp_row")
        rb0 = const.tile([64, S], BF16, tag="rb0")     # broadcast reciprocal
        rb1 = const.tile([64, S], BF16, tag="rb1")
        ident = const.tile([128, 128], BF16, tag="ident")
        make_identity(nc, ident[:])

        # ---- phase 1: projections + V transpose (all bf16)
        with tc.tile_pool(name="pps", bufs=3, space="PSUM") as pps:
            for nb in range(8) if "1" in _PHASES else ():
                for w_t, dst in ((wv_t, vT), (wq_t, qT), (wk_t, kT)):
                    acc = pps.tile([128, 512], F32, tag="acc")
                    for dc in range(4):
                        nc.tensor.matmul(acc[:], w_t[:, dc, :],
                                         xT_t[:, dc, nb * 512:(nb + 1) * 512],
                                         start=(dc == 0), stop=(dc == 3))
                    nc.scalar.activation(dst[:, nb * 512:(nb + 1) * 512], acc[:],
                                         mybir.ActivationFunctionType.Copy)
                # V transpose for this nb's 4 chunks (both units at once)
                for pci in (2 * nb, 2 * nb + 1):
                    tp = pps.tile([128, 2, 128], BF16, tag="tp")
                    for s in range(2):
                        ci = pci * 2 + s
                        nc.tensor.transpose(tp[:, s, :],
                                            vT[:, ci * 128:(ci + 1) * 128], ident[:])
                    nc.vector.tensor_copy(V[:, pci * 2:pci * 2 + 2, 0:64],
                                          tp[:, :, 0:64])
                    nc.vector.tensor_copy(V[:, pci * 2:pci * 2 + 2, 65:129],
                                          tp[:, :, 64:128])

        # ---- phase 2+3: attention, both units interleaved. The two heads'
        # score matmuls contract over disjoint 64-partition ranges (base
        # partition 0 / 64), so adjacent u0/u1 matmuls run concurrently in
        # different PE row groups. Interleaving also completes both units'
        # aoT blocks together; the output projection for the tokens covered
        # by each den block is emitted inline (its PSUM pool is co-resident,
        # so no pool-release barrier separates attention from projection).
        with tc.tile_pool(name="sps", bufs=2, space="PSUM") as sps, \
             tc.tile_pool(name="pvs", bufs=2, space="PSUM") as pvs, \
             tc.tile_pool(name="ops", bufs=2, space="PSUM") as ops, \
             tc.tile_pool(name="prp", bufs=8) as prp, \
             tc.tile_pool(name="yvp", bufs=4) as yvp:
            emitted = {}        # (u, ti) -> (pr2 tile, col offset)
            aoUs, rbs = (aoU0, aoU1), (rb0, rb1)

            def emit_pair(pid):
                a, b, pmid = PAIRS[pid]
                members = [m for m in (a, b) if m is not None]
                wtot = sum(TILES[m]["qw"] for m in members)
                sc2s = [sps.tile([128, 1024], F32, tag="sc2", name=f"sc2u{u}")
                        for u in range(2)]
                # interleave u0/u1 matmuls for PE row-group concurrency
                off = 0
                for m in members:
                    t = TILES[m]
                    for u in (0, 1):
                        ue = slice(u * 64, u * 64 + 64)
                        nc.tensor.matmul(sc2s[u][:, off:off + t["qw"]],
                                         kT[ue, t["c"] * 128:(t["c"] + 1) * 128],
                                         qT[ue, t["q0"]:t["q0"] + t["qw"]],
                                         start=True, stop=True)
                    off += t["qw"]
                for u in (0, 1):
                    pr2 = prp.tile([128, 1024], BF16, tag="pr2")
                    nc.scalar.activation(pr2[:, :wtot], sc2s[u][:, :wtot],
                                         mybir.ActivationFunctionType.Exp,
                                         scale=SCALE)
                    nc.vector.tensor_mul(pr2[:, :wtot], pr2[:, :wtot],
                                         mk_t[:, pmid, :wtot])
                    o = 0
                    for m in members:
                        emitted[(u, m)] = (pr2, o)
                        o += TILES[m]["qw"]

            if "2" in _PHASES:
                blk = 0
                otcn = 0
                for gi, g in enumerate(GROUPS):
                    # lookahead: emit pairs needed by this group and the next
                    for gg in (g,) + ((GROUPS[gi + 1],) if gi + 1 < len(GROUPS) else ()):
                        for _c, ti, _to, _oo, _ln in gg["segs"]:
                            if (0, ti) not in emitted:
                                emit_pair(TILES[ti]["pair"])
                    qw, q0 = g["qw"], g["q0"]
                    pv2 = pvs.tile([65, 2, 256], F32, tag="pv2")
                    nseg = len(g["segs"])
                    for u in (0, 1):
                        uv = slice(u * 65, u * 65 + 65)
                        for si, (c, ti, toff, ooff, ln) in enumerate(g["segs"]):
                            pr2, o = emitted[(u, ti)]
                            nc.tensor.matmul(pv2[:, u, ooff:ooff + ln],
                                             V[:, c, uv],
                                             pr2[:, o + toff:o + toff + ln],
                                             start=(si == 0), stop=(si == nseg - 1))
                        nc.vector.tensor_copy(aoUs[u][:, q0:q0 + qw],
                                              pv2[:, u, :qw])

                    # pipelined normalization: when this group closes a den
                    # block, reshape its den row to [64, L/64] via DRAM so the
                    # reciprocal runs on 64 lanes, then normalize the block
                    if blk < len(DBLOCKS) and gi == DBLOCKS[blk][0]:
                        _, qa, qb = DBLOCKS[blk]
                        L64 = (qb - qa) // 64
                        for u in (0, 1):
                            ue = slice(u * 64, u * 64 + 64)
                            aoU, rb = aoUs[u], rbs[u]
                            bi = u * len(DBLOCKS) + blk
                            nc.sync.dma_start(out=den_d[u][0:1, qa:qb],
                                              in_=aoU[64:65, qa:qb])
                            nc.sync.dma_start(
                                out=denT[:, bi, 0:L64],
                                in_=den_d[u][0:1, qa:qb].rearrange(
                                    "o (p f) -> (o p) f", p=64))
                            nc.vector.tensor_copy(denF[:, bi, 0:L64],
                                                  denT[:, bi, 0:L64])
                            nc.vector.reciprocal_approx_fast(rcpF[:, bi, 0:L64],
                                                             denF[:, bi, 0:L64])
                            nc.vector.tensor_copy(rcpB[:, bi, 0:L64],
                                                  rcpF[:, bi, 0:L64])
                            nc.sync.dma_start(
                                out=rcp_d[u][0:1, qa:qb].rearrange(
                                    "o (p f) -> (o p) f", p=64),
                                in_=rcpB[:, bi, 0:L64])
                            nc.sync.dma_start(out=rcp_row[:, u, qa:qb],
                                              in_=rcp_d[u][0:1, qa:qb])
                            nc.gpsimd.partition_broadcast(rb[:, qa:qb],
                                                          rcp_row[0:1, u, qa:qb])
                            nc.vector.tensor_mul(aoT[ue, qa:qb],
                                                 aoU[0:64, qa:qb], rb[:, qa:qb])
                        # output projection for token tiles this block covers
                        # (after BOTH units' aoT rows for the block are final)
                        if "3" in _PHASES:
                            while otcn * 128 + 128 <= qb:
                                acc = ops.tile([128, 512], F32, tag="oacc")
                                nc.tensor.matmul(
                                    acc[:], aoT[:, otcn * 128:(otcn + 1) * 128],
                                    wo_t[:], start=True, stop=True)
                                yv = yvp.tile([128, 512], BF16, tag="yv")
                                if otcn % 2 == 0:
                                    nc.vector.tensor_copy(yv[:], acc[:])
                                else:
                                    nc.scalar.activation(
                                        yv[:], acc[:],
                                        mybir.ActivationFunctionType.Copy)
                                nc.sync.dma_start(
                                    out=y_d[otcn * 128:(otcn + 1) * 128, :],
                                    in_=yv[:])
                                otcn += 1
                        blk += 1
                assert blk == len(DBLOCKS)
                assert otcn == 32 or "3" not in _PHASES
    nc.compile()
    return nc


def _get_module():
    if "nc" not in _NC_CACHE:
        _NC_CACHE["nc"] = _build_module()
    return _NC_CACHE["nc"]


# ---------------------------------------------------------------- host

BF = ml_dtypes.bfloat16
_MASKS_BF = np.ascontiguousarray(PMASKS.transpose(1, 0, 2)).astype(BF)


def _pack_pcm(a):
    """[512, M] -> [128, 4, M] with dim0 = (c*128 + p)."""
    m = a.shape[1]
    return np.ascontiguousarray(a.reshape(4, 128, m).transpose(1, 0, 2)).astype(BF)


def kernel(x, w_qkv, w_out):
    x = np.asarray(x, np.float32)
    w_qkv = np.asarray(w_qkv, np.float32)
    w_out = np.asarray(w_out, np.float32)
    nc = _get_module()

    xT = [_pack_pcm(np.ascontiguousarray(x[b].reshape(S, D).T)) for b in range(B)]
    w_outT = np.ascontiguousarray(w_out.T)

    in_maps = []
    for c in range(NCORES):
        b, h0 = c // 4, 2 * (c % 4)
        f = h0 * 64
        in_maps.append({
            "xT": xT[b],
            "wq": _pack_pcm(np.ascontiguousarray(w_qkv[f:f + 128].T)),
            "wk": _pack_pcm(np.ascontiguousarray(w_qkv[512 + f:512 + f + 128].T)),
            "wv": _pack_pcm(np.ascontiguousarray(w_qkv[1024 + f:1024 + f + 128].T)),
            "wo": np.ascontiguousarray(w_outT[f:f + 128]).astype(BF),
            "masks": _MASKS_BF,
        })
    res = run_bass_kernel_spmd(nc, in_maps, list(range(NCORES)), trace=TRACE,
                               tmpdir=TRACE_DIR)
    global LAST_RESULTS
    LAST_RESULTS = res
    y = np.zeros((B, S, D), np.float32)
    for c in range(NCORES):
        y[c // 4] += np.asarray(res.results[c]["y"], np.float32)
    return y.reshape(B, H, W, D)


# revision 29
# speedup vs baseline: 1.0565x; 1.0565x over previous
"""Neighbourhood attention block (7x7 clamped window) on 8 Trainium2 cores.

Sharding: (batch, head-pair) tensor parallel. Core c handles batch b = c//4
and heads (2*(c%4), 2*(c%4)+1). Each core computes q/k/v projections for its
two heads, neighbourhood attention, and a partial output projection
y_partial = ao @ w_out_slice^T (bf16). Host sums the 4 partials per batch.

Attention layout: scoresT [key, query] tiles so PV needs no transposes.
Keys are chunked 2 image rows (128 tokens) per chunk; each chunk is matched
against the 8 query rows that can see it (512 queries, one N=512 matmul).
Masking is multiplicative 0/1 after exp (bf16), so invalid keys contribute 0
to both PV numerator and the denominator (a ones-column appended to V).

v2 performance structure:
- everything bf16 on the matmul paths (same PE rate at N>=256, half DMA/DVE)
- x DMA split into 8 column chunks so projections start immediately
- score tiles PAIRED into one 2-bank PSUM tile; ONE exp ACTIVATE per pair
  (the ~352-cycle ACT fixed cost was 40% of scalar time)
- softmax normalization batched per unit: PV emits unnormalized numerators
  (copied to aoU) and denominators (copied to a [1, S] f32 row); one
  reciprocal_approx_fast + one partition_broadcast + 4 wide muls replace
  36 tiny single-partition RECIPROCALs (1.5us each!) and 36 broadcasts.
"""
import os
import numpy as np
import ml_dtypes
from contextlib import ExitStack

_PHASES = os.environ.get("KERNEL_PHASES", "123")  # debug bisect knob

import concourse.bass as bass
import concourse.bacc as bacc
import concourse.tile as tile
import concourse.mybir as mybir
from concourse.bass_utils import run_bass_kernel_spmd
from concourse.masks import make_identity

F32 = mybir.dt.float32
BF16 = mybir.dt.bfloat16

B, H, W, D = 2, 64, 64, 512
DH, NH = 64, 8
S = H * W              # 4096 tokens per batch
KER = 7
SCALE = DH ** -0.5     # 0.125
NCORES = 8

# ---------------------------------------------------------------- geometry

def _sh(r):            # clamped window start (rows); same formula for cols
    return min(max(r - KER // 2, 0), H - KER)


def _chunks_of_row(r):  # key chunks (2 rows each) seen by query row r
    s = _sh(r)
    return list(range(s // 2, (s + KER + 1) // 2))


def _build_plan():
    """Tiles: scoresT [128 keys of chunk c, qw queries at q0]. Groups: PV
    accumulations [65, qw] covering disjoint query ranges."""
    tiles = []          # dict(c, q0, qw)
    for c in range(32):
        q0r = min(max(2 * c - 3, 0), 56)
        tiles.append(dict(c=c, q0=q0r * 64, qw=512))
    for c in (2, 3):        # query rows 0..2 miss these chunks' main windows
        tiles.append(dict(c=c, q0=0, qw=192))
    for c in (28, 29):      # query rows 61..63
        tiles.append(dict(c=c, q0=61 * 64, qw=192))

    # rows covered by each tile, for resolution
    def covers(t, r):
        return t["q0"] <= r * 64 and (r + 1) * 64 <= t["q0"] + t["qw"]

    groups = []         # dict(q0, qw, rows)
    groups.append(dict(rows=[0]))
    groups.append(dict(rows=[1, 2]))
    for k in range(14):
        r0 = 4 * k + 3
        groups.append(dict(rows=[r0, r0 + 1, r0 + 2, r0 + 3]))
    groups.append(dict(rows=[59, 60, 61, 62]))
    groups.append(dict(rows=[63]))

    for g in groups:
        rows = g["rows"]
        g["q0"] = rows[0] * 64
        g["qw"] = len(rows) * 64
        # chunk -> contiguous row subrange of this group needing it
        chunk_rows = {}
        for r in rows:
            for c in _chunks_of_row(r):
                a, b = chunk_rows.get(c, (r, r))
                chunk_rows[c] = (min(a, r), max(b, r))
        mms = []        # (c, row_a, row_b_inclusive, width)
        for c, (ra, rb) in sorted(chunk_rows.items()):
            mms.append((c, ra, rb, (rb - ra + 1) * 64))
        mms.sort(key=lambda m: -m[3])   # widest (full-group) first for start=True
        assert mms[0][3] == g["qw"], (g, mms)
        # resolve each (c, row range) to probs-tile segments
        segs = []       # (c, tile_i, tile_off, out_off, length)
        for c, ra, rb, _w in mms:
            r = ra
            while r <= rb:
                cand = [i for i, t in enumerate(tiles) if t["c"] == c and covers(t, r)]
                assert cand, (g, c, r)
                ti = cand[0]
                t = tiles[ti]
                # extend run while same tile covers
                r2 = r
                while r2 + 1 <= rb and covers(t, r2 + 1):
                    r2 += 1
                segs.append((c, ti, r * 64 - t["q0"], r * 64 - g["q0"],
                             (r2 - r + 1) * 64))
                r = r2 + 1
        g["segs"] = segs

    # sanity: every (query row, chunk) incidence consumed exactly once
    seen = set()
    for g in groups:
        for c, ti, toff, ooff, ln in g["segs"]:
            for r in range((g["q0"] + ooff) // 64, (g["q0"] + ooff + ln) // 64):
                key = (r, c)
                assert key not in seen, key
                seen.add(key)
    for r in range(H):
        for c in _chunks_of_row(r):
            assert (r, c) in seen, (r, c)

    # masks per tile (0/1), deduped
    starts = np.minimum(np.maximum(np.arange(H) - KER // 2, 0), H - KER)
    valid = (np.arange(H)[None, :] >= starts[:, None]) & \
            (np.arange(H)[None, :] < starts[:, None] + KER)   # [q pos, k pos]

    def tile_mask(t):
        ktok = t["c"] * 128 + np.arange(128)
        qtok = t["q0"] + np.arange(t["qw"])
        m = np.zeros((128, 512), np.float32)
        m[:, :t["qw"]] = (valid[qtok[None, :] // 64, ktok[:, None] // 64]
                          & valid[qtok[None, :] % 64, ktok[:, None] % 64])
        return m

    mask_list, mask_ids = [], {}
    for t in tiles:
        m = tile_mask(t)
        key = m.tobytes()
        if key not in mask_ids:
            mask_ids[key] = len(mask_list)
            mask_list.append(m)
        t["mask_id"] = mask_ids[key]

    # first-need order of tiles over the group sequence, and static pairing
    # of consecutive tiles (each pair shares one 2-bank PSUM tile + one exp
    # + one paired-mask multiply)
    order, seen_t = [], set()
    for g in groups:
        for _c, ti, _to, _oo, _ln in g["segs"]:
            if ti not in seen_t:
                seen_t.add(ti)
                order.append(ti)
    pairs = []          # list of (ti_a, ti_b or None)
    for i in range(0, len(order), 2):
        pairs.append((order[i], order[i + 1] if i + 1 < len(order) else None))
    for pid, (a, b) in enumerate(pairs):
        tiles[a]["pair"] = pid
        tiles[a]["poff"] = 0
        if b is not None:
            tiles[b]["pair"] = pid
            tiles[b]["poff"] = tiles[a]["qw"]

    # paired masks [128, 1024], deduped across pairs
    pm_list, pm_ids = [], {}
    for pid, (a, b) in enumerate(pairs):
        m = np.zeros((128, 1024), np.float32)
        wa = tiles[a]["qw"]
        m[:, :wa] = mask_list[tiles[a]["mask_id"]][:, :wa]
        if b is not None:
            wb = tiles[b]["qw"]
            m[:, wa:wa + wb] = mask_list[tiles[b]["mask_id"]][:, :wb]
        key = m.tobytes()
        if key not in pm_ids:
            pm_ids[key] = len(pm_list)
            pm_list.append(m)
        pairs[pid] = (a, b, pm_ids[key])

    # den-normalization blocks: runs of groups whose cumulative query end is
    # a multiple of 64 and roughly 1k wide -> pipelined reshape/recip
    blocks, start, prev_end = [], 0, 0
    for gi, g in enumerate(groups):
        end = g["q0"] + g["qw"]
        if end - start >= 960:
            blocks.append((gi, start, end))
            start = end
    if start < groups[-1]["q0"] + groups[-1]["qw"]:
        blocks.append((len(groups) - 1, start, groups[-1]["q0"] + groups[-1]["qw"]))
    assert all((e - s) % 64 == 0 for _, s, e in blocks)
    return tiles, groups, np.stack(mask_list), pairs, np.stack(pm_list), blocks


TILES, GROUPS, MASKS, PAIRS, PMASKS, DBLOCKS = _build_plan()
NMASK = len(MASKS)
NPMASK = len(PMASKS)
MAXBW = max((e - s) // 64 for _, s, e in DBLOCKS)

# ---------------------------------------------------------------- device

_NC_CACHE = {}
TRACE = False          # set True (e.g. from test.py) to capture an NTFF profile
TRACE_DIR = None       # persistent dir for NTFF/perfetto artifacts
LAST_RESULTS = None    # BassKernelResults of the most recent kernel() call


def _build_module():
    nc = bacc.Bacc("TRN2", target_bir_lowering=False, debug=False,
                   num_devices=NCORES)
    xT_d = nc.dram_tensor("xT", [128, 4, S], BF16, kind="ExternalInput")
    wq_d = nc.dram_tensor("wq", [128, 4, 128], BF16, kind="ExternalInput")
    wk_d = nc.dram_tensor("wk", [128, 4, 128], BF16, kind="ExternalInput")
    wv_d = nc.dram_tensor("wv", [128, 4, 128], BF16, kind="ExternalInput")
    wo_d = nc.dram_tensor("wo", [128, 512], BF16, kind="ExternalInput")
    mk_d = nc.dram_tensor("masks", [128, NPMASK, 1024], BF16, kind="ExternalInput")
    y_d = nc.dram_tensor("y", [S, D], BF16, kind="ExternalOutput")
    # DRAM scratch for the softmax-denominator partition reshape
    den_d = [nc.dram_tensor(f"den{u}", [1, S], BF16, kind="Internal") for u in range(2)]
    rcp_d = [nc.dram_tensor(f"rcp{u}", [1, S], BF16, kind="Internal") for u in range(2)]

    with tile.TileContext(nc) as tc, ExitStack() as ctx:
        const = ctx.enter_context(tc.tile_pool(name="const", bufs=1))
        wq_t = const.tile([128, 4, 128], BF16, tag="wq")
        nc.sync.dma_start(out=wq_t[:], in_=wq_d[:, :, :])
        wk_t = const.tile([128, 4, 128], BF16, tag="wk")
        nc.sync.dma_start(out=wk_t[:], in_=wk_d[:, :, :])
        wv_t = const.tile([128, 4, 128], BF16, tag="wv")
        nc.sync.dma_start(out=wv_t[:], in_=wv_d[:, :, :])
        wo_t = const.tile([128, 512], BF16, tag="wo")
        nc.sync.dma_start(out=wo_t[:], in_=wo_d[:, :])
        # masks on the gpsimd-triggered queue so they don't sit behind x
        mk_t = const.tile([128, NPMASK, 1024], BF16, tag="mk")
        nc.gpsimd.dma_start(out=mk_t[:], in_=mk_d[:, :, :])
        # x in 8 column chunks so proj can start on chunk 0 asap
        xT_t = const.tile([128, 4, S], BF16, tag="xT")
        for nb in range(8):
            nc.sync.dma_start(out=xT_t[:, :, nb * 512:(nb + 1) * 512],
                              in_=xT_d[:, :, nb * 512:(nb + 1) * 512])

        qT = const.tile([128, S], BF16, tag="qT")      # [2 heads x 64e, tok]
        kT = const.tile([128, S], BF16, tag="kT")
        vT = const.tile([128, S], BF16, tag="vT")
        # V: [tok_in_chunk, chunk, 130]: cols 0:64 u0-e, 64 ones, 65:129 u1-e, 129 ones
        V = const.tile([128, 32, 130], BF16, tag="V")
        nc.gpsimd.memset(V[:], 1.0)
        aoT = const.tile([128, S], BF16, tag="aoT")    # normalized, outproj lhsT
        # unnormalized numerators (rows 0:64) + denominator (row 64)
        aoU0 = const.tile([65, S], BF16, tag="aoU0")
        aoU1 = const.tile([65, S], BF16, tag="aoU1")
        NBLK = len(DBLOCKS)
        denT = const.tile([64, 2 * NBLK, MAXBW], BF16, tag="denT")  # den reshaped
        denF = const.tile([64, 2 * NBLK, MAXBW], F32, tag="denF")
        rcpF = const.tile([64, 2 * NBLK, MAXBW], F32, tag="rcpF")
        rcpB = const.tile([64, 2 * NBLK, MAXBW], BF16, tag="rcpB")
        rcp_row = const.tile([1, 2, S], BF16, tag="rcp_row")
        rb0 = const.tile([64, S], BF16, tag="rb0")     # broadcast reciprocal
        rb1 = const.tile([64, S], BF16, tag="rb1")
        ident = const.tile([128, 128], BF16, tag="ident")
        make_identity(nc, ident[:])

        # ---- phase 1: projections + V transpose (all bf16)
        with tc.tile_pool(name="pps", bufs=3, space="PSUM") as pps:
            for nb in range(8) if "1" in _PHASES else ():
                for w_t, dst in ((wv_t, vT), (wq_t, qT), (wk_t, kT)):
                    acc = pps.tile([128, 512], F32, tag="acc")
                    for dc in range(4):
                        nc.tensor.matmul(acc[:], w_t[:, dc, :],
                                         xT_t[:, dc, nb * 512:(nb + 1) * 512],
                                         start=(dc == 0), stop=(dc == 3))
                    nc.scalar.activation(dst[:, nb * 512:(nb + 1) * 512], acc[:],
                                         mybir.ActivationFunctionType.Copy)
                # V transpose for this nb's 4 chunks (both units at once)
                for pci in (2 * nb, 2 * nb + 1):
                    tp = pps.tile([128, 2, 128], BF16, tag="tp")
                    for s in range(2):
                        ci = pci * 2 + s
                        nc.tensor.transpose(tp[:, s, :],
                                            vT[:, ci * 128:(ci + 1) * 128], ident[:])
                    nc.vector.tensor_copy(V[:, pci * 2:pci * 2 + 2, 0:64],
                                          tp[:, :, 0:64])
                    nc.vector.tensor_copy(V[:, pci * 2:pci * 2 + 2, 65:129],
                                          tp[:, :, 64:128])

        # ---- phase 2+3: attention, both units interleaved. The two heads'
        # score matmuls contract over disjoint 64-partition ranges (base
        # partition 0 / 64), so adjacent u0/u1 matmuls run concurrently in
        # different PE row groups. Interleaving also completes both units'
        # aoT blocks together; the output projection for the tokens covered
        # by each den block is emitted inline (its PSUM pool is co-resident,
        # so no pool-release barrier separates attention from projection).
        with tc.tile_pool(name="sps", bufs=2, space="PSUM") as sps, \
             tc.tile_pool(name="pvs", bufs=2, space="PSUM") as pvs, \
             tc.tile_pool(name="ops", bufs=2, space="PSUM") as ops, \
             tc.tile_pool(name="prp", bufs=8) as prp, \
             tc.tile_pool(name="yvp", bufs=4) as yvp:
            emitted = {}        # (u, ti) -> (pr2 tile, col offset)
            aoUs, rbs = (aoU0, aoU1), (rb0, rb1)

            def emit_pair(pid):
                a, b, pmid = PAIRS[pid]
                members = [m for m in (a, b) if m is not None]
                wtot = sum(TILES[m]["qw"] for m in members)
                sc2s = [sps.tile([128, 1024], F32, tag="sc2", name=f"sc2u{u}")
                        for u in range(2)]
                # interleave u0/u1 matmuls for PE row-group concurrency
                off = 0
                for m in members:
                    t = TILES[m]
                    for u in (0, 1):
                        ue = slice(u * 64, u * 64 + 64)
                        nc.tensor.matmul(sc2s[u][:, off:off + t["qw"]],
                                         kT[ue, t["c"] * 128:(t["c"] + 1) * 128],
                                         qT[ue, t["q0"]:t["q0"] + t["qw"]],
                                         start=True, stop=True)
                    off += t["qw"]
                for u in (0, 1):
                    pr2 = prp.tile([128, 1024], BF16, tag="pr2")
                    nc.scalar.activation(pr2[:, :wtot], sc2s[u][:, :wtot],
                                         mybir.ActivationFunctionType.Exp,
                                         scale=SCALE)
                    nc.vector.tensor_mul(pr2[:, :wtot], pr2[:, :wtot],
                                         mk_t[:, pmid, :wtot])
                    o = 0
                    for m in members:
                        emitted[(u, m)] = (pr2, o)
                        o += TILES[m]["qw"]

            if "2" in _PHASES:
                blk = 0
                otcn = 0
                for gi, g in enumerate(GROUPS):
                    # lookahead: emit pairs needed by this group and the next
                    for gg in (g,) + ((GROUPS[gi + 1],) if gi + 1 < len(GROUPS) else ()):
                        for _c, ti, _to, _oo, _ln in gg["segs"]:
                            if (0, ti) not in emitted:
                                emit_pair(TILES[ti]["pair"])
                    qw, q0 = g["qw"], g["q0"]
                    pv2 = pvs.tile([65, 2, 256], F32, tag="pv2")
                    nseg = len(g["segs"])
                    for u in (0, 1):
                        uv = slice(u * 65, u * 65 + 65)
                        for si, (c, ti, toff, ooff, ln) in enumerate(g["segs"]):
                            pr2, o = emitted[(u, ti)]
                            nc.tensor.matmul(pv2[:, u, ooff:ooff + ln],
                                             V[:, c, uv],
                                             pr2[:, o + toff:o + toff + ln],
                                             start=(si == 0), stop=(si == nseg - 1))
                        nc.vector.tensor_copy(aoUs[u][:, q0:q0 + qw],
                                              pv2[:, u, :qw])

                    # pipelined normalization: when this group closes a den
                    # block, reshape its den row to [64, L/64] via DRAM so the
                    # reciprocal runs on 64 lanes, then normalize the block
                    if blk < len(DBLOCKS) and gi == DBLOCKS[blk][0]:
                        _, qa, qb = DBLOCKS[blk]
                        L64 = (qb - qa) // 64
                        for u in (0, 1):
                            ue = slice(u * 64, u * 64 + 64)
                            aoU, rb = aoUs[u], rbs[u]
                            bi = u * len(DBLOCKS) + blk
                            nc.sync.dma_start(out=den_d[u][0:1, qa:qb],
                                              in_=aoU[64:65, qa:qb])
                            nc.sync.dma_start(
                                out=denT[:, bi, 0:L64],
                                in_=den_d[u][0:1, qa:qb].rearrange(
                                    "o (p f) -> (o p) f", p=64))
                            nc.vector.tensor_copy(denF[:, bi, 0:L64],
                                                  denT[:, bi, 0:L64])
                            nc.vector.reciprocal_approx_fast(rcpF[:, bi, 0:L64],
                                                             denF[:, bi, 0:L64])
                            nc.vector.tensor_copy(rcpB[:, bi, 0:L64],
                                                  rcpF[:, bi, 0:L64])
                            nc.sync.dma_start(
                                out=rcp_d[u][0:1, qa:qb].rearrange(
                                    "o (p f) -> (o p) f", p=64),
                                in_=rcpB[:, bi, 0:L64])
                            nc.sync.dma_start(out=rcp_row[:, u, qa:qb],
                                              in_=rcp_d[u][0:1, qa:qb])
                            nc.gpsimd.partition_broadcast(rb[:, qa:qb],
                                                          rcp_row[0:1, u, qa:qb])
                            nc.vector.tensor_mul(aoT[ue, qa:qb],
                                                 aoU[0:64, qa:qb], rb[:, qa:qb])
                        # output projection for token tiles covered by the
                        # PREVIOUS block — its ~6us den DMA chain has had a
                        # whole block of group compute to finish, so these
                        # matmuls never stall the in-order PE queue
                        if "3" in _PHASES and blk > 0:
                            while otcn * 128 + 128 <= DBLOCKS[blk - 1][2]:
                                acc = ops.tile([128, 512], F32, tag="oacc")
                                nc.tensor.matmul(
                                    acc[:], aoT[:, otcn * 128:(otcn + 1) * 128],
                                    wo_t[:], start=True, stop=True)
                                yv = yvp.tile([128, 512], BF16, tag="yv")
                                if otcn % 2 == 0:
                                    nc.vector.tensor_copy(yv[:], acc[:])
                                else:
                                    nc.scalar.activation(
                                        yv[:], acc[:],
                                        mybir.ActivationFunctionType.Copy)
                                nc.sync.dma_start(
                                    out=y_d[otcn * 128:(otcn + 1) * 128, :],
                                    in_=yv[:])
                                otcn += 1
                        blk += 1
                assert blk == len(DBLOCKS)
                # flush the remaining output projection (last block's tokens)
                for tcn in range(otcn, 32) if "3" in _PHASES else ():
                    acc = ops.tile([128, 512], F32, tag="oacc")
                    nc.tensor.matmul(acc[:], aoT[:, tcn * 128:(tcn + 1) * 128],
                                     wo_t[:], start=True, stop=True)
                    yv = yvp.tile([128, 512], BF16, tag="yv")
                    if tcn % 2 == 0:
                        nc.vector.tensor_copy(yv[:], acc[:])
                    else:
                        nc.scalar.activation(yv[:], acc[:],
                                             mybir.ActivationFunctionType.Copy)
                    nc.sync.dma_start(out=y_d[tcn * 128:(tcn + 1) * 128, :],
                                      in_=yv[:])
    nc.compile()
    return nc


def _get_module():
    if "nc" not in _NC_CACHE:
        _NC_CACHE["nc"] = _build_module()
    return _NC_CACHE["nc"]


# ---------------------------------------------------------------- host

BF = ml_dtypes.bfloat16
_MASKS_BF = np.ascontiguousarray(PMASKS.transpose(1, 0, 2)).astype(BF)


def _pack_pcm(a):
    """[512, M] -> [128, 4, M] with dim0 = (c*128 + p)."""
    m = a.shape[1]
    return np.ascontiguousarray(a.reshape(4, 128, m).transpose(1, 0, 2)).astype(BF)


def kernel(x, w_qkv, w_out):
    x = np.asarray(x, np.float32)
    w_qkv = np.asarray(w_qkv, np.float32)
    w_out = np.asarray(w_out, np.float32)
    nc = _get_module()

    xT = [_pack_pcm(np.ascontiguousarray(x[b].reshape(S, D).T)) for b in range(B)]
    w_outT = np.ascontiguousarray(w_out.T)

    in_maps = []
    for c in range(NCORES):
        b, h0 = c // 4, 2 * (c % 4)
        f = h0 * 64
        in_maps.append({
            "xT": xT[b],
            "wq": _pack_pcm(np.ascontiguousarray(w_qkv[f:f + 128].T)),
            "wk": _pack_pcm(np.ascontiguousarray(w_qkv[512 + f:512 + f + 128].T)),
            "wv": _pack_pcm(np.ascontiguousarray(w_qkv[1024 + f:1024 + f + 128].T)),
            "wo": np.ascontiguousarray(w_outT[f:f + 128]).astype(BF),
            "masks": _MASKS_BF,
        })
    res = run_bass_kernel_spmd(nc, in_maps, list(range(NCORES)), trace=TRACE,
                               tmpdir=TRACE_DIR)
    global LAST_RESULTS
    LAST_RESULTS = res
    y = np.zeros((B, S, D), np.float32)
    for c in range(NCORES):
        y[c // 4] += np.asarray(res.results[c]["y"], np.float32)
    return y.reshape(B, H, W, D)
